# revision 2
# baseline (speedup 1.0000x reference)
"""BloomAttention (B=2, S=1024, H=4096, 32 heads, head_dim=128) on 8 TRN2
NeuronCores — tensor-parallel over heads (4 heads per core).

Strategy (per core, SPMD — one Bass program, per-core data):
  * STRIDED head sharding: core c owns heads {c, c+8, c+16, c+24} (slots
    0..3). Slot s therefore holds a head with ALiBi slope <= slopes[8s+7],
    which bounds the attention window per slot at COMPILE time:
    slot0 <= 0.25 -> ~100 positions, slot1 <= 0.0625 -> ~400, slots 2/3
    effectively full-causal. k-tiles whose entire ALiBi factor is below
    exp(-25) are skipped (contribution ~1e-10 of softmax mass).
  * hidden_states pre-transposed on host to hidT [B, H, S], bf16
    (replicated); w_qkv column-sliced per core (head-strided), w_out
    row-sliced, both bf16.
  * QKV: chunked-K GEMMs (4 chunks x 8 K-tiles), fp32 PSUM chains, fp32
    SBUF accumulation across chunks, final chunk fused with bf16 downcast.
  * Attention per head-pair (slots 0+1, 2+3), 256-col q panels:
      scoresT[k,q] = kT.T @ qT  (both heads packed in one PSUM bank)
      P = exp(scoresT) [Scalar] * EADP [DVE/GpSimd]  (EADP = per-pair
      exp(slope*(j-i)) window tables with the causal mask as exact zeros)
      ctxT += v.T @ P  (PSUM-accumulated, head pair packed in one bank)
      den: P tiles pair-summed on DVE (bf16), then ones.T @ presum chained
      in PSUM — halves the PE den cost vs per-step den matmuls.
      Diagonal k-tile: only the upper 128 q-cols are unmasked, so scores/
      exp/mul/ctx/den all run at half width there.
      ctx = ctxT * reciprocal_approx_fast(den)  [DVE]
  * Projection (w_out resident in SBUF) interleaved per 512-col block as
    soon as its ctx tiles complete; bf16 partials [H, B*S] written out;
    the host sums the 8 partials, transposes back, and adds
    b_out + b_v @ w_out (v-bias commutes through the softmax-linear ops).
  * q/k biases applied on-chip during PSUM eviction; attention scaling
    folded into the q eviction.
"""

import math
import numpy as np
from contextlib import ExitStack

import concourse.bass as bass
import concourse.tile as tile
import concourse.mybir as mybir
from concourse import bacc
from concourse.bass_utils import run_bass_kernel_spmd

f32 = mybir.dt.float32
f32r = mybir.dt.float32r
bf16 = mybir.dt.bfloat16
AF = mybir.ActivationFunctionType
ALU = mybir.AluOpType

B, S, H = 2, 1024, 4096
TOTAL_HEADS = 32
N_CORES = 8
HPC = TOTAL_HEADS // N_CORES      # heads per core (= slots)
HD = HPC * 128                    # per-core head feature width
OFF = 384                         # D-table offset
W = OFF + S                       # D-table width
MASK_FILL = -1.0e5
N_CHUNKS = 4

# ALiBi window tile-keep sets per slot (see docstring). Slot s keeps
# k-tile j of panel qc iff 128j+127 >= 256qc - T/slope_min(slot),
# T=25, slope_min = [0.25, 0.0625, 0.015625, 0.0039].
KEEP = [
    {0: [0, 1], 1: [1, 2, 3], 2: [3, 4, 5], 3: [5, 6, 7]},            # slot0
    {0: [0, 1], 1: [0, 1, 2, 3], 2: [0, 1, 2, 3, 4, 5],
     3: [2, 3, 4, 5, 6, 7]},                                          # slot1
    {qc: list(range(2 * qc + 2)) for qc in range(4)},                 # slot2
    {qc: list(range(2 * qc + 2)) for qc in range(4)},                 # slot3
]


def _build_nc(n_devices=N_CORES, repeat=1, nonce=1):
    hpc = HPC
    NH_T = H // 128               # 32 K-tiles over hidden dim
    CH = NH_T // N_CHUNKS         # 8 K-tiles per chunk
    SB = S // 512                 # 2 proj/QKV seq blocks
    ST = S // 128                 # 8 k-tiles over seq
    QC = S // 256                 # 4 attention q panels
    OG = H // 512                 # 8 proj output groups
    scaling = float(128 ** -0.5)

    nc = bacc.Bacc("TRN2", target_bir_lowering=False, debug=False,
                   num_devices=n_devices)
    hidT = nc.dram_tensor("hidT", [B, H, S], bf16, kind="ExternalInput").ap()
    wq = nc.dram_tensor("wq", [H, HD], bf16, kind="ExternalInput").ap()
    wk = nc.dram_tensor("wk", [H, HD], bf16, kind="ExternalInput").ap()
    wv = nc.dram_tensor("wv", [H, HD], bf16, kind="ExternalInput").ap()
    wo = nc.dram_tensor("wo", [HD, H], bf16, kind="ExternalInput").ap()
    bq = nc.dram_tensor("bq", [128, hpc], f32, kind="ExternalInput").ap()
    bk = nc.dram_tensor("bk", [128, hpc], f32, kind="ExternalInput").ap()
    slp = nc.dram_tensor("slp", [128, hpc], f32, kind="ExternalInput").ap()
    outp = nc.dram_tensor("outp", [H, B * S], bf16, kind="ExternalOutput").ap()
    nonce_t = nc.dram_tensor("nonce", [1, int(nonce)], f32,
                             kind="ExternalInput").ap()
    del nonce_t

    with tile.TileContext(nc) as tc:
        with ExitStack() as ctx:
            const = ctx.enter_context(tc.tile_pool(name="const", bufs=1))
            hidp = ctx.enter_context(tc.tile_pool(name="hidp", bufs=2 * CH))
            wsp = ctx.enter_context(tc.tile_pool(name="wsp", bufs=2 * CH + 6))
            accp = ctx.enter_context(tc.tile_pool(name="accp", bufs=8))
            qkp = ctx.enter_context(tc.tile_pool(name="qkp", bufs=2 * hpc))
            vnp = ctx.enter_context(tc.tile_pool(name="vnp", bufs=ST))
            pp = ctx.enter_context(tc.tile_pool(name="pp", bufs=6))
            prp = ctx.enter_context(tc.tile_pool(name="prp", bufs=3))
            rp = ctx.enter_context(tc.tile_pool(name="rp", bufs=4))
            ctxp = ctx.enter_context(tc.tile_pool(name="ctxp", bufs=hpc * SB))
            osp = ctx.enter_context(tc.tile_pool(name="osp", bufs=6))
            psp = ctx.enter_context(tc.tile_pool(name="psp", bufs=4, space="PSUM"))

            ps_ctr = [0]

            def flow_tile(cols=512):
                ps_ctr[0] += 1
                return psp.tile([128, cols], f32, tag="flow", bufs=4,
                                name=f"psf_{ps_ctr[0]}")

            def held_tile():
                ps_ctr[0] += 1
                return psp.tile([128, 512], f32, tag="held", bufs=4,
                                name=f"psh_{ps_ctr[0]}")

            # ---- constants ----
            Dext = const.tile([128, W], f32, tag="dext")
            nc.gpsimd.iota(Dext[:], base=OFF, channel_multiplier=1,
                           pattern=[[-1, W]],
                           allow_small_or_imprecise_dtypes=True)
            nc.gpsimd.affine_select(Dext[:], Dext[:], base=-OFF,
                                    channel_multiplier=-1, pattern=[[1, W]],
                                    compare_op=ALU.is_ge, fill=MASK_FILL)
            ones_f = const.tile([128, 128], f32, tag="onesf")
            nc.gpsimd.memset(ones_f[:], 1.0)
            ones = const.tile([128, 128], bf16, tag="ones")
            nc.vector.tensor_copy(ones[:], ones_f[:])
            bq_t = const.tile([128, hpc], f32, tag="bq")
            nc.sync.dma_start(bq_t[:], bq[:])
            bk_t = const.tile([128, hpc], f32, tag="bk")
            nc.sync.dma_start(bk_t[:], bk[:])
            slp_t = const.tile([128, hpc], f32, tag="slp")
            nc.sync.dma_start(slp_t[:], slp[:])

            # Combined per-pair ALiBi tables: EADP[p][o][:, 0:256] covers
            # slot 2p, [:, 256:512] covers slot 2p+1, for k/q tile offset
            # o = 2*qc - j (table col t <-> Dext col OFF + 128*o + t').
            # pair0 offsets 2..4 only ever feed half-width (slot1) steps,
            # so only their upper half is built.
            EADP = {}
            for p in range(2):
                sa, sb_ = 2 * p, 2 * p + 1
                omax = 4 if p == 0 else 6
                for o in range(omax + 1):
                    t = const.tile([128, 512], bf16, tag="eadp", bufs=12,
                                   name=f"eadp_{p}_{o}")
                    d = Dext[:, OFF + 128 * o: OFF + 128 * o + 256]
                    if p == 1 or o <= 1:
                        nc.scalar.activation(t[:, 0:256], d, AF.Exp,
                                             scale=slp_t[:, sa:sa + 1])
                    nc.scalar.activation(t[:, 256:512], d, AF.Exp,
                                         scale=slp_t[:, sb_:sb_ + 1])
                    EADP[(p, o)] = t
            wo_t = {}

            for bi in range(B * repeat):
                b = bi % B
                # ================= QKV =================
                qk_final = {
                    "q": [qkp.tile([128, S], bf16, tag="qkT",
                                   name=f"qT_{bi}_{i}") for i in range(hpc)],
                    "k": [qkp.tile([128, S], bf16, tag="qkT",
                                   name=f"kT_{bi}_{i}") for i in range(hpc)],
                }
                v_final = [vnp.tile([128, HD], bf16, tag="vn",
                                    name=f"vN_{bi}_{i}") for i in range(ST)]
                qk_acc = {
                    "q": [accp.tile([128, S], f32, tag="qkacc", bufs=8,
                                    name=f"qA_{bi}_{i}") for i in range(hpc)],
                    "k": [accp.tile([128, S], f32, tag="qkacc", bufs=8,
                                    name=f"kA_{bi}_{i}") for i in range(hpc)],
                }
                v_acc = [accp.tile([128, HD], f32, tag="vacc", bufs=ST,
                                   name=f"vA_{bi}_{i}") for i in range(ST)]

                for hc in range(N_CHUNKS):
                    hts = list(range(hc * CH, (hc + 1) * CH))
                    hid_t = {}
                    w_t = {}
                    for ht in hts:
                        t = hidp.tile([128, S], bf16, tag="hidt",
                                      name=f"hid_{bi}_{ht}")
                        nc.sync.dma_start(
                            t[:], hidT[b, ht * 128:(ht + 1) * 128, :])
                        hid_t[ht] = t
                        wqt = wsp.tile([128, HD], bf16, tag="w",
                                       name=f"wq_{bi}_{ht}")
                        nc.sync.dma_start(
                            wqt[:], wq[ht * 128:(ht + 1) * 128, :])
                        w_t[("q", ht)] = wqt

                    # Q then K: output-stationary chains per (head, sub)
                    for which, wsrc, bias_t, sc in (
                        ("q", wq, bq_t, scaling), ("k", wk, bk_t, 1.0)):
                        if which == "k":
                            for ht in hts:
                                wkt = wsp.tile([128, HD], bf16, tag="w",
                                               name=f"wk_{bi}_{ht}")
                                nc.sync.dma_start(
                                    wkt[:],
                                    wsrc[ht * 128:(ht + 1) * 128, :])
                                w_t[("k", ht)] = wkt
                        for head in range(hpc):
                            for sub in range(SB):
                                ps = flow_tile()
                                for i, ht in enumerate(hts):
                                    nc.tensor.matmul(
                                        ps[:],
                                        w_t[(which, ht)][:, head * 128:(head + 1) * 128],
                                        hid_t[ht][:, sub * 512:(sub + 1) * 512],
                                        start=(i == 0), stop=(i == CH - 1))
                                acc = qk_acc[which][head][:, sub * 512:(sub + 1) * 512]
                                if hc == 0:
                                    nc.vector.tensor_scalar(
                                        out=acc, in0=ps[:],
                                        scalar1=sc, scalar2=bias_t[:, head:head + 1],
                                        op0=ALU.mult, op1=ALU.add)
                                elif hc < N_CHUNKS - 1:
                                    nc.vector.scalar_tensor_tensor(
                                        out=acc, in0=ps[:],
                                        scalar=sc, in1=acc,
                                        op0=ALU.mult, op1=ALU.add)
                                else:
                                    fin = qk_final[which][head]
                                    nc.vector.scalar_tensor_tensor(
                                        out=fin[:, sub * 512:(sub + 1) * 512],
                                        in0=ps[:], scalar=sc, in1=acc,
                                        op0=ALU.mult, op1=ALU.add)

                    # V: output-stationary chains per s-tile
                    for ht in hts:
                        wvt = wsp.tile([128, HD], bf16, tag="w",
                                       name=f"wv_{bi}_{ht}")
                        nc.sync.dma_start(
                            wvt[:], wv[ht * 128:(ht + 1) * 128, :])
                        w_t[("v", ht)] = wvt
                    for st in range(ST):
                        ps = flow_tile(HD)
                        for i, ht in enumerate(hts):
                            nc.tensor.matmul(
                                ps[:],
                                hid_t[ht][:, st * 128:(st + 1) * 128],
                                w_t[("v", ht)][:],
                                start=(i == 0), stop=(i == CH - 1))
                        if hc == 0:
                            nc.vector.tensor_copy(v_acc[st][:], ps[:])
                        elif hc < N_CHUNKS - 1:
                            nc.vector.tensor_add(v_acc[st][:], ps[:], v_acc[st][:])
                        else:
                            nc.vector.tensor_add(v_final[st][:], ps[:], v_acc[st][:])

                # w_out resident load, once (overlaps batch-0 attention)
                if bi == 0:
                    for og in range(OG):
                        for f in range(hpc):
                            t = const.tile([128, 512], bf16, tag="wo",
                                           bufs=OG * hpc, name=f"wo_{og}_{f}")
                            nc.sync.dma_start(
                                t[:],
                                wo[f * 128:(f + 1) * 128,
                                   og * 512:(og + 1) * 512])
                            wo_t[(og, f)] = t

                # ====== attention (256-col q panels, windowed) + proj ======
                ctx_tiles = {h: [ctxp.tile([128, 512], bf16, tag="ctx",
                                           name=f"ctx_{bi}_{h}_{sb}")
                                 for sb in range(SB)] for h in range(hpc)}
                for qc in range(QC):
                    q0 = qc * 256
                    J = 2 * qc + 2
                    for p in range(2):
                        sa, sb_ = 2 * p, 2 * p + 1
                        KA = KEEP[sa][qc]
                        KB = KEEP[sb_][qc]
                        # step plan: D = diagonal (half-width packed),
                        # F = both slots, H = hi-slot only
                        plan = [(j, 'D' if j == J - 1 else
                                 ('F' if j in KA else 'H')) for j in KB]
                        nF = sum(1 for _, k in plan if k == 'F')
                        nH = sum(1 for _, k in plan if k == 'H')
                        n_den = (nF // 2 + nF % 2 + nH // 2 + nH % 2 + 2)
                        ps_ctx = held_tile()
                        ps_den = held_tile()
                        cs = {sa: ps_ctx[:, 0:256], sb_: ps_ctx[:, 256:512]}
                        ds = {sa: ps_den[:, 0:256], sb_: ps_den[:, 256:512]}
                        den_i = [0]

                        def den_mm(out_ap, mov_ap, nden=n_den, di=den_i,
                                   pd=ps_den):
                            nc.tensor.matmul(
                                out_ap, ones[:], mov_ap,
                                start=(di[0] == 0), stop=(di[0] == nden - 1),
                                skip_group_check=True)
                            di[0] += 1

                        ctx_first = [True]

                        def ctx_mm(region, vslice, mov_ap, stop):
                            nc.tensor.matmul(
                                region, vslice, mov_ap,
                                start=ctx_first[0], stop=stop,
                                skip_group_check=True)
                            ctx_first[0] = False

                        pendF = []
                        pendH = []
                        for j, kind in plan:
                            o = 2 * qc - j
                            if kind == 'F':
                                ps_s = flow_tile()
                                for hi, h in enumerate((sa, sb_)):
                                    nc.tensor.matmul(
                                        ps_s[:, hi * 256:(hi + 1) * 256],
                                        qk_final["k"][h][:, j * 128:(j + 1) * 128],
                                        qk_final["q"][h][:, q0:q0 + 256],
                                        start=(hi == 0), stop=True,
                                        skip_group_check=(hi == 1))
                                P = pp.tile([128, 512], bf16, tag="P")
                                nc.scalar.activation(P[:], ps_s[:], AF.Exp)
                                nc.vector.tensor_mul(
                                    P[:], P[:], EADP[(p, o)][:])
                                for hi, h in enumerate((sa, sb_)):
                                    ctx_mm(cs[h],
                                           v_final[j][:, h * 128:(h + 1) * 128],
                                           P[:, hi * 256:(hi + 1) * 256],
                                           stop=False)
                                pendF.append(P)
                                if len(pendF) == 2:
                                    pr = prp.tile([128, 512], bf16, tag="pr")
                                    nc.vector.tensor_add(
                                        pr[:], pendF[0][:], pendF[1][:])
                                    den_mm(ps_den[:, 0:512], pr[:])
                                    pendF = []
                            elif kind == 'H':
                                ps_s = flow_tile()
                                nc.tensor.matmul(
                                    ps_s[:, 256:512],
                                    qk_final["k"][sb_][:, j * 128:(j + 1) * 128],
                                    qk_final["q"][sb_][:, q0:q0 + 256],
                                    start=True, stop=True)
                                P = pp.tile([128, 512], bf16, tag="P")
                                nc.scalar.activation(
                                    P[:, 256:512], ps_s[:, 256:512], AF.Exp)
                                nc.gpsimd.tensor_mul(
                                    P[:, 256:512], P[:, 256:512],
                                    EADP[(p, o)][:, 256:512])
                                ctx_mm(cs[sb_],
                                       v_final[j][:, sb_ * 128:(sb_ + 1) * 128],
                                       P[:, 256:512], stop=False)
                                pendH.append(P)
                                if len(pendH) == 2:
                                    pr = prp.tile([128, 256], bf16, tag="prh")
                                    nc.vector.tensor_add(
                                        pr[:], pendH[0][:, 256:512],
                                        pendH[1][:, 256:512])
                                    den_mm(ps_den[:, 256:512], pr[:])
                                    pendH = []
                            else:  # 'D' — diagonal, upper 128 q-cols only
                                ps_s = flow_tile()
                                for hi, h in enumerate((sa, sb_)):
                                    nc.tensor.matmul(
                                        ps_s[:, hi * 128:(hi + 1) * 128],
                                        qk_final["k"][h][:, j * 128:(j + 1) * 128],
                                        qk_final["q"][h][:, q0 + 128:q0 + 256],
                                        start=(hi == 0), stop=True,
                                        skip_group_check=(hi == 1))
                                P = pp.tile([128, 512], bf16, tag="P")
                                nc.scalar.activation(
                                    P[:, 0:256], ps_s[:, 0:256], AF.Exp)
                                # diag rel-offsets equal EADP[p][0] cols
                                # [0:128] (slot a) / [256:384] (slot b)
                                nc.gpsimd.tensor_mul(
                                    P[:, 0:128], P[:, 0:128],
                                    EADP[(p, 0)][:, 0:128])
                                nc.gpsimd.tensor_mul(
                                    P[:, 128:256], P[:, 128:256],
                                    EADP[(p, 0)][:, 256:384])
                                ctx_mm(ps_ctx[:, 128:256],
                                       v_final[j][:, sa * 128:(sa + 1) * 128],
                                       P[:, 0:128], stop=True)
                                ctx_mm(ps_ctx[:, 384:512],
                                       v_final[j][:, sb_ * 128:(sb_ + 1) * 128],
                                       P[:, 128:256], stop=True)
                                den_mm(ps_den[:, 128:256], P[:, 0:128])
                                den_mm(ps_den[:, 384:512], P[:, 128:256])
                        # flush leftover presum pends
                        if pendF:
                            den_mm(ps_den[:, 0:512], pendF[0][:])
                        if pendH:
                            den_mm(ps_den[:, 256:512], pendH[0][:, 256:512])
                        assert den_i[0] == n_den

                        for h in (sa, sb_):
                            recip = rp.tile([128, 256], f32, tag="recip")
                            nc.vector.reciprocal_approx_fast(recip[:], ds[h])
                            half = (qc % 2) * 256
                            nc.vector.tensor_mul(
                                ctx_tiles[h][qc // 2][:, half:half + 256],
                                cs[h], recip[:])

                    # proj for seq block sb as soon as its ctx completes
                    if qc % 2 == 1:
                        sb = qc // 2
                        for og in range(OG):
                            for ot in range(4):
                                ps_o = flow_tile()
                                for f in range(hpc):
                                    nc.tensor.matmul(
                                        ps_o[:],
                                        wo_t[(og, f)][:, ot * 128:(ot + 1) * 128],
                                        ctx_tiles[f][sb][:],
                                        start=(f == 0), stop=(f == hpc - 1))
                                ost = osp.tile([128, 512], bf16, tag="ost")
                                nc.vector.tensor_copy(ost[:], ps_o[:])
                                r0 = og * 512 + ot * 128
                                c0 = b * S + sb * 512
                                nc.sync.dma_start(
                                    outp[r0:r0 + 128, c0:c0 + 512], ost[:])

    nc.compile()
    return nc


def _alibi_slopes(total_heads):
    closest = 2 ** math.floor(math.log2(total_heads))
    base = 2 ** (-(2 ** (-(math.log2(closest) - 3))))
    powers = np.arange(1, 1 + closest, dtype=np.float32)
    slopes = np.power(base, powers).astype(np.float32)
    if closest != total_heads:
        extra_base = 2 ** (-(2 ** (-(math.log2(2 * closest) - 3))))
        num_rem = min(closest, total_heads - closest)
        extra = np.arange(1, 1 + 2 * num_rem, 2, dtype=np.float32)
        slopes = np.concatenate(
            [slopes, np.power(extra_base, extra).astype(np.float32)])
    return slopes


_NC_CACHE = {}


def _get_nc():
    if "nc" not in _NC_CACHE:
        _NC_CACHE["nc"] = _build_nc()
    return _NC_CACHE["nc"]


def make_in_maps(hidden_states, w_qkv, b_qkv, w_out):
    """Build the 8 per-core input dicts (strided head sharding)."""
    import ml_dtypes
    bf = ml_dtypes.bfloat16
    scaling = np.float32(128 ** -0.5)
    hidT = np.ascontiguousarray(
        hidden_states.transpose(0, 2, 1)).astype(bf)
    slopes = _alibi_slopes(TOTAL_HEADS)
    nonce = np.zeros((1, 1), np.float32)
    in_maps = []
    for core in range(N_CORES):
        heads = [core + 8 * s for s in range(HPC)]
        wq_c = np.concatenate(
            [w_qkv[:, h * 128:(h + 1) * 128] for h in heads], axis=1)
        wk_c = np.concatenate(
            [w_qkv[:, H + h * 128:H + (h + 1) * 128] for h in heads], axis=1)
        wv_c = np.concatenate(
            [w_qkv[:, 2 * H + h * 128:2 * H + (h + 1) * 128] for h in heads],
            axis=1)
        wo_c = np.concatenate(
            [w_out[h * 128:(h + 1) * 128, :] for h in heads], axis=0)
        bq_v = np.stack(
            [b_qkv[h * 128:(h + 1) * 128] for h in heads], axis=1)  # [128,hpc]
        bk_v = np.stack(
            [b_qkv[H + h * 128:H + (h + 1) * 128] for h in heads], axis=1)
        sl = slopes[heads]
        in_maps.append(dict(
            hidT=hidT,
            wq=np.ascontiguousarray(wq_c).astype(bf),
            wk=np.ascontiguousarray(wk_c).astype(bf),
            wv=np.ascontiguousarray(wv_c).astype(bf),
            wo=np.ascontiguousarray(wo_c).astype(bf),
            bq=np.ascontiguousarray((bq_v * scaling).astype(np.float32)),
            bk=np.ascontiguousarray(bk_v.astype(np.float32)),
            slp=np.ascontiguousarray(
                np.broadcast_to(sl[None, :], (128, HPC)).astype(np.float32)),
            nonce=nonce,
        ))
    return in_maps


def finish_output(partials, b_qkv, w_out, b_out):
    """Host-side all-reduce over cores + layout fix + bias."""
    total = np.zeros((H, B * S), dtype=np.float64)
    for p in partials:
        total += p.astype(np.float64)
    bias_vec = (b_qkv[2 * H:].astype(np.float64) @ w_out.astype(np.float64)
                + b_out.astype(np.float64))
    out = total.reshape(H, B, S).transpose(1, 2, 0) + bias_vec[None, None, :]
    return np.ascontiguousarray(out.astype(np.float32))


def kernel(hidden_states, w_qkv, b_qkv, w_out, b_out):
    hidden_states = np.asarray(hidden_states, dtype=np.float32)
    w_qkv = np.asarray(w_qkv, dtype=np.float32)
    b_qkv = np.asarray(b_qkv, dtype=np.float32)
    w_out = np.asarray(w_out, dtype=np.float32)
    b_out = np.asarray(b_out, dtype=np.float32)

    nc = _get_nc()
    in_maps = make_in_maps(hidden_states, w_qkv, b_qkv, w_out)
    res = run_bass_kernel_spmd(nc, in_maps, core_ids=list(range(N_CORES)))
    return finish_output([res.results[c]["outp"] for c in range(N_CORES)],
                         b_qkv, w_out, b_out)
